# revision 30
# baseline (speedup 1.0000x reference)
"""MoE top-2 routing kernel for 8 Trainium2 NeuronCores.

Strategy (expert-parallel dispatch):
  - Router (logits/softmax/top-2/aux-loss) computed host-side in fp32 numpy;
    the routing decision IS the sharding decision, so it has to precede the
    device dispatch anyway. Margins between 2nd/3rd expert are >=1.7e-5 for
    this input scale, far above fp32 noise, so host routing matches the
    reference's routing deterministically.
  - Tokens are gathered per expert (counts ~2048 each), padded to a fixed
    capacity, and each of the 8 cores runs Y_e = X_e @ W[e] as a dense
    [cap,1024]x[1024,1024] matmul with a custom SBUF-resident Tile kernel.
  - Host combines: out[n] = sum_k top_w[n,k] * (Y[e_k, pos] + b[e_k]).

Env knobs (for benchmarking from test.py; defaults are the shipped config):
  BASS_MOE_DTYPE  = bfloat16 | float32 | float32r  (matmul input dtype)
  BASS_MOE_SPLIT3 = 1  bf16 hi/lo split: 3 bf16 matmuls, ~fp32 accuracy
  BASS_MOE_TRACE  = 1 to run with NTFF tracing (needs antenv.axon_hooks shim)
"""

import os

import numpy as np
import ml_dtypes

N_TOKENS = 8192
D_IN = 1024
D_OUT = 1024
N_EXPERTS = 8
TOP_K = 2
LB_WEIGHT = 0.01

CAP_MIN = 2176  # >= max expert load for the reference input (2121), mult of 128

MM_DTYPE = os.environ.get("BASS_MOE_DTYPE", "bfloat16")
SPLIT3 = os.environ.get("BASS_MOE_SPLIT3", "0") == "1"
IMPL = os.environ.get("BASS_MOE_IMPL", "raw")  # raw | tile

_prog_cache = {}
last_results = None  # BassKernelResults of the most recent device run


def _np_dtype(dt_name):
    return {
        "float32": np.float32,
        "float32r": np.float32,
        "bfloat16": ml_dtypes.bfloat16,
    }[dt_name]


def _build_program_raw(cap, dt_name):
    """Raw-Bass (manual semaphores) version: avoids Tile's ~6us preamble and
    ~12us exit drain. Same packed DRAM layouts as the Tile version.

    Pipeline: sync issues input DMAs (x[m=0], W k-slices, x[m>=1]) on the
    FIFO HWDGE ring; PE runs 16 accumulating matmuls per token tile into a
    rotating set of 3 PSUM bank pairs; vector/scalar copy the two output
    halves to a 4-deep SBUF ring; gpsimd streams results out.
    """
    key = (cap, dt_name, "raw")
    if key in _prog_cache:
        return _prog_cache[key]

    import concourse.bass as bass
    import concourse.mybir as mybir

    mm_dt = getattr(mybir.dt, dt_name)
    f32 = mybir.dt.float32
    P = 128
    NF = 512
    KO = D_IN // P
    MT = cap // P
    NT = D_OUT // NF
    assert NT == 2

    nc = bass.Bass(target_bir_lowering=False)
    xh = nc.dram_tensor("xh", [P, MT * KO * P], mm_dt, kind="ExternalInput").ap()
    wh = nc.dram_tensor("wh", [P, KO * D_OUT], mm_dt, kind="ExternalInput").ap()
    yh = nc.dram_tensor("yh", [P, MT * D_OUT], f32, kind="ExternalOutput").ap()

    NOB = 4  # output SBUF ring depth
    NPS = 3  # psum bank pairs

    from contextlib import ExitStack

    with ExitStack() as stack:
        mm0 = stack.enter_context(nc.semaphore("mm0"))
        mm1 = stack.enter_context(nc.semaphore("mm1"))
        cp0 = stack.enter_context(nc.semaphore("cp0"))
        cp1 = stack.enter_context(nc.semaphore("cp1"))
        # per-DMA semaphores: single 0->16 increment each, so waits never
        # depend on cross-DMA completion order. x tiles m=0..4 load singly
        # (fine-grained pacing while W streams); m>=5 load in pairs.
        x_chunks = [(m, m + 1) for m in range(MT)]
        o_chunks = [(m, min(m + 2, MT)) for m in range(0, MT, 2)]
        xs_sem = [stack.enter_context(nc.semaphore(f"xs{i}")) for i in range(len(x_chunks))]
        xs0b = stack.enter_context(nc.semaphore("xs0b"))
        warm_sem = stack.enter_context(nc.semaphore("warm_sem"))
        ws_sem = [stack.enter_context(nc.semaphore(f"ws{k}")) for k in range(KO)]
        os_sem = [stack.enter_context(nc.semaphore(f"os{i}")) for i in range(len(o_chunks))]
        x_chunk_of = {}
        for i, (a, b) in enumerate(x_chunks):
            for m in range(a, b):
                x_chunk_of[m] = i
        o_chunk_of = {}
        for i, (a, b) in enumerate(o_chunks):
            for m in range(a, b):
                o_chunk_of[m] = i
        x_sb = stack.enter_context(nc.sbuf_tensor("x_sb", [P, MT * KO * P], mm_dt)).ap()
        w_sb = stack.enter_context(nc.sbuf_tensor("w_sb", [P, KO * D_OUT], mm_dt)).ap()
        ot_sb = stack.enter_context(nc.sbuf_tensor("ot_sb", [P, NOB * D_OUT], f32)).ap()
        dummy = stack.enter_context(nc.sbuf_tensor("warm_sb", [P, NF], mybir.dt.bfloat16)).ap()
        ps = stack.enter_context(nc.psum_tensor("ps", [P, NPS * NT * NF], f32)).ap()
        ps_warm = stack.enter_context(nc.psum_tensor("ps_warm", [P, NF], f32)).ap()
        mm_sems = [mm0, mm1]

        def x_tile(m, k):
            return x_sb[:, (m * KO + k) * P : (m * KO + k + 1) * P]

        def w_tile(k, n):
            c = k * D_OUT + n * NF
            return w_sb[:, c : c + NF]

        def ps_tile(m, n):
            c = ((m % NPS) * NT + n) * NF
            return ps[:, c : c + NF]

        def ot_tile(m, n=None):
            c = (m % NOB) * D_OUT
            if n is None:
                return ot_sb[:, c : c + D_OUT]
            return ot_sb[:, c + n * NF : c + (n + 1) * NF]

        with nc.Block() as block:

            @block.sync
            def _(sync):
                # single FIFO ring: x[0] first, then W per-k (paces the first
                # group's k-accumulation), then remaining x chunks.
                # x[0] and W[k=0] go in k-split halves so the very first
                # matmul's operands land as early as possible.
                sync.dma_start(
                    out=x_sb[:, 0 : 2 * P], in_=xh[:, 0 : 2 * P]
                ).then_inc(xs_sem[0], 16)
                sync.dma_start(
                    out=w_sb[:, 0:D_OUT], in_=wh[:, 0:D_OUT]
                ).then_inc(ws_sem[0], 16)
                sync.dma_start(
                    out=x_sb[:, 2 * P : KO * P], in_=xh[:, 2 * P : KO * P]
                ).then_inc(xs0b, 16)
                for k in range(1, KO):
                    sync.dma_start(
                        out=w_sb[:, k * D_OUT : (k + 1) * D_OUT],
                        in_=wh[:, k * D_OUT : (k + 1) * D_OUT],
                    ).then_inc(ws_sem[k], 16)
                for i, (a, b) in enumerate(x_chunks):
                    if i == 0:
                        continue
                    sync.dma_start(
                        out=x_sb[:, a * KO * P : b * KO * P],
                        in_=xh[:, a * KO * P : b * KO * P],
                    ).then_inc(xs_sem[i], 16)
                # output DMAs on this HWDGE ring too (ring is idle by then)
                for i, (a, b) in enumerate(o_chunks):
                    sync.wait_ge(cp0, b)
                    sync.wait_ge(cp1, b)
                    sync.dma_start(
                        out=yh[:, a * D_OUT : b * D_OUT],
                        in_=ot_sb[:, (a % NOB) * D_OUT : (a % NOB + b - a) * D_OUT],
                    ).then_inc(os_sem[i], 16)
                for i in range(len(o_chunks)):
                    sync.wait_ge(os_sem[i], 16)

            @block.tensor
            def _(tensor):
                # HAM warmup: ~10 throwaway matmuls on a scratch bank while the
                # first input DMAs are in flight, so real matmuls start at the
                # un-throttled PE clock (and the wait time isn't dead time).
                tensor.wait_ge(warm_sem, 1)
                for wi in range(10):
                    tensor.matmul(
                        ps_warm[:, :NF],
                        lhsT=dummy[:, :P],
                        rhs=dummy[:, :],
                        start=(wi == 0),
                        stop=(wi == 9),
                    )
                for m in range(MT):
                    if m >= NPS:  # psum pair reuse: copies of group m-NPS done
                        tensor.wait_ge(cp0, m - NPS + 1)
                        tensor.wait_ge(cp1, m - NPS + 1)
                    if m == 0 or x_chunk_of[m] != x_chunk_of[m - 1]:
                        tensor.wait_ge(xs_sem[x_chunk_of[m]], 16)
                    for k in range(KO):
                        if m == 0:
                            tensor.wait_ge(ws_sem[k], 16)
                            if k == 2:
                                tensor.wait_ge(xs0b, 16)
                        for n in range(NT):
                            mm = tensor.matmul(
                                ps_tile(m, n),
                                lhsT=x_tile(m, k),
                                rhs=w_tile(k, n),
                                start=(k == 0),
                                stop=(k == KO - 1),
                            )
                            if k == KO - 1:
                                mm.then_inc(mm_sems[n], 1)

            @block.gpsimd
            def _(gpsimd):
                gpsimd.memset(dummy[:, :], 0).then_inc(warm_sem, 1)

            @block.vector
            def _(vector):
                for m in range(MT):
                    vector.wait_ge(mm0, m + 1)
                    if m >= NOB:
                        vector.wait_ge(os_sem[o_chunk_of[m - NOB]], 16)
                    vector.tensor_copy(ot_tile(m, 0), ps_tile(m, 0)).then_inc(cp0, 1)

            @block.scalar
            def _(scalar):
                for m in range(MT):
                    scalar.wait_ge(mm1, m + 1)
                    if m >= NOB:
                        scalar.wait_ge(os_sem[o_chunk_of[m - NOB]], 16)
                    scalar.copy(ot_tile(m, 1), ps_tile(m, 1)).then_inc(cp1, 1)



    _prog_cache[key] = nc
    return nc


def _build_program(cap, dt_name, split3):
    if IMPL == "raw" and not split3:
        return _build_program_raw(cap, dt_name)
    key = (cap, dt_name, split3)
    if key in _prog_cache:
        return _prog_cache[key]

    import concourse.mybir as mybir
    import concourse.tile as tile
    from concourse import bacc
    from concourse.bass import ds, ts

    mm_dt = getattr(mybir.dt, dt_name)
    f32 = mybir.dt.float32
    P = 128
    NF = 512  # matmul free dim / PSUM bank
    KO = D_IN // P  # 8 contraction tiles
    MT = cap // P  # token tiles
    NT = D_OUT // NF  # 2 output column tiles

    # Host-side packed layouts (one contiguous run per partition per DMA):
    #   xh[p, m*KO*P + k*P + c] = X[m*P + c, k*P + p]   (lhsT tiles, row-major)
    #   wh[p, k*D_OUT + n]      = W[k*P + p, n]
    #   yh[p, m*D_OUT + n]      = Y[m*P + p, n]
    nc = bacc.Bacc("TRN2", target_bir_lowering=False, debug=False)
    xh = nc.dram_tensor("xh", [P, MT * KO * P], mm_dt, kind="ExternalInput").ap()
    wh = nc.dram_tensor("wh", [P, KO * D_OUT], mm_dt, kind="ExternalInput").ap()
    if split3:
        xhl = nc.dram_tensor("xhl", [P, MT * KO * P], mm_dt, kind="ExternalInput").ap()
        whl = nc.dram_tensor("whl", [P, KO * D_OUT], mm_dt, kind="ExternalInput").ap()
    yh = nc.dram_tensor("yh", [P, MT * D_OUT], f32, kind="ExternalOutput").ap()

    with tile.TileContext(nc) as tc:
        with (
            tc.tile_pool(name="const", bufs=1) as cpool,
            tc.tile_pool(name="psum", bufs=3, space="PSUM") as ppool,
            tc.tile_pool(name="outp", bufs=4) as opool,
        ):
            w_sb = cpool.tile([P, KO * D_OUT], mm_dt, tag="w")
            x_sb = cpool.tile([P, MT, KO * P], mm_dt, tag="x")
            if split3:
                wl_sb = cpool.tile([P, KO * D_OUT], mm_dt, tag="wl")
                xl_sb = cpool.tile([P, MT, KO * P], mm_dt, tag="xl")
            # x[m=0] first (first matmul group), then W per-k (pipelines the
            # first group's k-accumulation), then the remaining x tiles.
            nc.sync.dma_start(out=x_sb[:, 0, :], in_=xh[:, ds(0, KO * P)])
            if split3:
                nc.sync.dma_start(out=xl_sb[:, 0, :], in_=xhl[:, ds(0, KO * P)])
            for k in range(KO):
                nc.sync.dma_start(
                    out=w_sb[:, ds(k * D_OUT, D_OUT)], in_=wh[:, ds(k * D_OUT, D_OUT)]
                )
                if split3:
                    nc.sync.dma_start(
                        out=wl_sb[:, ds(k * D_OUT, D_OUT)],
                        in_=whl[:, ds(k * D_OUT, D_OUT)],
                    )
            for m in range(1, MT):
                nc.sync.dma_start(
                    out=x_sb[:, m, :], in_=xh[:, ds(m * KO * P, KO * P)]
                )
                if split3:
                    nc.sync.dma_start(
                        out=xl_sb[:, m, :], in_=xhl[:, ds(m * KO * P, KO * P)]
                    )

            for m in range(MT):
                ps = [
                    ppool.tile([P, NF], f32, tag=f"ps{n}", name=f"ps{n}_{m}")
                    for n in range(NT)
                ]
                for k in range(KO):
                    pairs = [(x_sb, w_sb)]
                    if split3:
                        pairs = [(x_sb, w_sb), (x_sb, wl_sb), (xl_sb, w_sb)]
                    for pi, (xs, ws) in enumerate(pairs):
                        for n in range(NT):
                            nc.tensor.matmul(
                                ps[n][:],
                                lhsT=xs[:, m, ds(k * P, P)],
                                rhs=ws[:, ds(k * D_OUT + n * NF, NF)],
                                start=(k == 0 and pi == 0),
                                stop=(k == KO - 1 and pi == len(pairs) - 1),
                            )
                ot = opool.tile([P, D_OUT], f32, tag="ot")
                for n in range(NT):
                    if n % 2 == 0:
                        nc.vector.tensor_copy(ot[:, ds(n * NF, NF)], ps[n][:])
                    else:
                        nc.scalar.copy(ot[:, ds(n * NF, NF)], ps[n][:])
                nc.gpsimd.dma_start(out=yh[:, ds(m * D_OUT, D_OUT)], in_=ot[:])
    nc.compile()
    _prog_cache[key] = nc
    return nc


def _route(x, Wr, br):
    """fp32 router identical to the reference's routing decisions."""
    logits = x @ Wr.T + br  # [N, E] fp32
    m = logits.max(axis=-1, keepdims=True)
    e = np.exp(logits - m)
    probs = e / e.sum(axis=-1, keepdims=True)  # fp32
    # descending stable sort == jax.lax.top_k tie-breaking (lowest index wins)
    top_i = np.argsort(-probs, axis=-1, kind="stable")[:, :TOP_K]
    top_w = np.take_along_axis(probs, top_i, axis=-1)
    top_w = top_w / top_w.sum(axis=-1, keepdims=True)
    mean_prob = probs.mean(axis=0, dtype=np.float64)
    aux = np.float32(np.mean((mean_prob - 1.0 / N_EXPERTS) ** 2) * LB_WEIGHT)
    return probs, top_i, top_w, aux


def kernel(x, W, b, Wr, br):
    global last_results
    x = np.ascontiguousarray(np.asarray(x, dtype=np.float32))
    W = np.asarray(W, dtype=np.float32)
    b = np.asarray(b, dtype=np.float32)
    Wr = np.asarray(Wr, dtype=np.float32)
    br = np.asarray(br, dtype=np.float32)

    probs, top_i, top_w, aux = _route(x, Wr, br)

    # --- dispatch: gather tokens per expert, pad to capacity ---
    np_dt = _np_dtype(MM_DTYPE)
    idx = [np.where((top_i == e).any(axis=1))[0] for e in range(N_EXPERTS)]
    counts = np.array([len(i) for i in idx])
    cap = max(CAP_MIN, int(-(-counts.max() // 128) * 128))

    P = 128
    KO = D_IN // P
    MT = cap // P

    def pack_x(xg):  # [cap, D_IN] -> [P, MT*KO*P]: xh[p, (m,k,c)] = xg[mP+c, kP+p]
        return np.ascontiguousarray(
            xg.reshape(MT, P, KO, P).transpose(3, 0, 2, 1).reshape(P, MT * KO * P)
        )

    def pack_w(we):  # [D_IN, D_OUT] -> [P, KO*D_OUT]: wh[p, (k,n)] = we[kP+p, n]
        return np.ascontiguousarray(
            we.reshape(KO, P, D_OUT).transpose(1, 0, 2).reshape(P, KO * D_OUT)
        )

    in_maps = []
    for e in range(N_EXPERTS):
        xg = np.zeros((cap, D_IN), dtype=np.float32)
        xg[: counts[e]] = x[idx[e]]
        m = {"xh": pack_x(xg.astype(np_dt)), "wh": pack_w(W[e].astype(np_dt))}
        if SPLIT3:
            xlo = (xg - xg.astype(np_dt).astype(np.float32)).astype(np_dt)
            wlo = (W[e] - W[e].astype(np_dt).astype(np.float32)).astype(np_dt)
            m["xhl"] = pack_x(xlo)
            m["whl"] = pack_w(wlo)
        in_maps.append(m)

    # --- device: Y_e = X_e @ W[e] on core e ---
    from concourse.bass_utils import run_bass_kernel_spmd

    nc = _build_program(cap, MM_DTYPE, SPLIT3)
    trace = os.environ.get("BASS_MOE_TRACE", "0") == "1"
    kwargs = {}
    if trace:
        kwargs = dict(trace=True, tmpdir=os.environ.get("BASS_MOE_TRACE_DIR"))
    res = run_bass_kernel_spmd(nc, in_maps, list(range(N_EXPERTS)), **kwargs)
    last_results = res
    # yh[p, (m,n)] = Y[m*P+p, n]  ->  Y[cap, D_OUT]
    Y = np.stack(
        [
            res.results[i]["yh"]
            .reshape(P, MT, D_OUT)
            .transpose(1, 0, 2)
            .reshape(cap, D_OUT)
            for i in range(N_EXPERTS)
        ]
    )  # [E, cap, O] f32

    # --- combine: out[n] = sum_k w_k * (Y[e_k, pos_k] + b[e_k]) ---
    pos = np.zeros((N_EXPERTS, N_TOKENS), dtype=np.int64)
    for e in range(N_EXPERTS):
        pos[e, idx[e]] = np.arange(counts[e])
    tok = np.arange(N_TOKENS)
    out = np.zeros((N_TOKENS, D_OUT), dtype=np.float32)
    for k in range(TOP_K):
        ek = top_i[:, k]
        rows = Y[ek, pos[ek, tok]]  # [N, O]
        out += top_w[:, k : k + 1] * (rows + b[ek])

    return out, aux


# revision 31
# speedup vs baseline: 1.0288x; 1.0288x over previous
"""MoE top-2 routing kernel for 8 Trainium2 NeuronCores.

Strategy (expert-parallel dispatch):
  - Router (logits/softmax/top-2/aux-loss) computed host-side in fp32 numpy;
    the routing decision IS the sharding decision, so it has to precede the
    device dispatch anyway. Margins between 2nd/3rd expert are >=1.7e-5 for
    this input scale, far above fp32 noise, so host routing matches the
    reference's routing deterministically.
  - Tokens are gathered per expert (counts ~2048 each), padded to a fixed
    capacity, and each of the 8 cores runs Y_e = X_e @ W[e] as a dense
    [cap,1024]x[1024,1024] matmul with a custom SBUF-resident Tile kernel.
  - Host combines: out[n] = sum_k top_w[n,k] * (Y[e_k, pos] + b[e_k]).

Env knobs (for benchmarking from test.py; defaults are the shipped config):
  BASS_MOE_DTYPE  = bfloat16 | float32 | float32r  (matmul input dtype)
  BASS_MOE_SPLIT3 = 1  bf16 hi/lo split: 3 bf16 matmuls, ~fp32 accuracy
  BASS_MOE_TRACE  = 1 to run with NTFF tracing (needs antenv.axon_hooks shim)
"""

import os

import numpy as np
import ml_dtypes

N_TOKENS = 8192
D_IN = 1024
D_OUT = 1024
N_EXPERTS = 8
TOP_K = 2
LB_WEIGHT = 0.01

CAP_MIN = 2176  # >= max expert load for the reference input (2121), mult of 128

MM_DTYPE = os.environ.get("BASS_MOE_DTYPE", "bfloat16")
SPLIT3 = os.environ.get("BASS_MOE_SPLIT3", "0") == "1"
IMPL = os.environ.get("BASS_MOE_IMPL", "raw")  # raw | tile
N_WARM = int(os.environ.get("BASS_MOE_WARM", "10"))

_prog_cache = {}
last_results = None  # BassKernelResults of the most recent device run


def _np_dtype(dt_name):
    return {
        "float32": np.float32,
        "float32r": np.float32,
        "bfloat16": ml_dtypes.bfloat16,
    }[dt_name]


def _build_program_raw(cap, dt_name):
    """Raw-Bass (manual semaphores) version: avoids Tile's ~6us preamble and
    ~12us exit drain. Same packed DRAM layouts as the Tile version.

    Pipeline: sync issues input DMAs (x[m=0], W k-slices, x[m>=1]) on the
    FIFO HWDGE ring; PE runs 16 accumulating matmuls per token tile into a
    rotating set of 3 PSUM bank pairs; vector/scalar copy the two output
    halves to a 4-deep SBUF ring; gpsimd streams results out.
    """
    key = (cap, dt_name, "raw", N_WARM)
    if key in _prog_cache:
        return _prog_cache[key]

    import concourse.bass as bass
    import concourse.mybir as mybir

    mm_dt = getattr(mybir.dt, dt_name)
    f32 = mybir.dt.float32
    P = 128
    NF = 512
    KO = D_IN // P
    MT = cap // P
    NT = D_OUT // NF
    assert NT == 2

    nc = bass.Bass(target_bir_lowering=False)
    xh = nc.dram_tensor("xh", [P, MT * KO * P], mm_dt, kind="ExternalInput").ap()
    wh = nc.dram_tensor("wh", [P, KO * D_OUT], mm_dt, kind="ExternalInput").ap()
    yh = nc.dram_tensor("yh", [P, MT * D_OUT], f32, kind="ExternalOutput").ap()

    NOB = 4  # output SBUF ring depth
    NPS = 3  # psum bank pairs

    from contextlib import ExitStack

    with ExitStack() as stack:
        mm0 = stack.enter_context(nc.semaphore("mm0"))
        mm1 = stack.enter_context(nc.semaphore("mm1"))
        cp0 = stack.enter_context(nc.semaphore("cp0"))
        cp1 = stack.enter_context(nc.semaphore("cp1"))
        # per-DMA semaphores: single 0->16 increment each, so waits never
        # depend on cross-DMA completion order. x tiles m=0..4 load singly
        # (fine-grained pacing while W streams); m>=5 load in pairs.
        x_chunks = [(m, m + 1) for m in range(MT)]
        o_chunks = [(m, min(m + 2, MT)) for m in range(0, MT, 2)]
        xs_sem = [stack.enter_context(nc.semaphore(f"xs{i}")) for i in range(len(x_chunks))]
        xs0b = stack.enter_context(nc.semaphore("xs0b"))
        warm_sem = stack.enter_context(nc.semaphore("warm_sem"))
        ws_sem = [stack.enter_context(nc.semaphore(f"ws{k}")) for k in range(KO)]
        os_sem = [stack.enter_context(nc.semaphore(f"os{i}")) for i in range(len(o_chunks))]
        x_chunk_of = {}
        for i, (a, b) in enumerate(x_chunks):
            for m in range(a, b):
                x_chunk_of[m] = i
        o_chunk_of = {}
        for i, (a, b) in enumerate(o_chunks):
            for m in range(a, b):
                o_chunk_of[m] = i
        x_sb = stack.enter_context(nc.sbuf_tensor("x_sb", [P, MT * KO * P], mm_dt)).ap()
        w_sb = stack.enter_context(nc.sbuf_tensor("w_sb", [P, KO * D_OUT], mm_dt)).ap()
        ot_sb = stack.enter_context(nc.sbuf_tensor("ot_sb", [P, NOB * D_OUT], f32)).ap()
        dummy = stack.enter_context(nc.sbuf_tensor("warm_sb", [P, NF], mybir.dt.bfloat16)).ap()
        ps = stack.enter_context(nc.psum_tensor("ps", [P, NPS * NT * NF], f32)).ap()
        ps_warm = stack.enter_context(nc.psum_tensor("ps_warm", [P, NF], f32)).ap()
        mm_sems = [mm0, mm1]

        def x_tile(m, k):
            return x_sb[:, (m * KO + k) * P : (m * KO + k + 1) * P]

        def w_tile(k, n):
            c = k * D_OUT + n * NF
            return w_sb[:, c : c + NF]

        def ps_tile(m, n):
            c = ((m % NPS) * NT + n) * NF
            return ps[:, c : c + NF]

        def ot_tile(m, n=None):
            c = (m % NOB) * D_OUT
            if n is None:
                return ot_sb[:, c : c + D_OUT]
            return ot_sb[:, c + n * NF : c + (n + 1) * NF]

        with nc.Block() as block:

            @block.sync
            def _(sync):
                # single FIFO ring: x[0] first, then W per-k (paces the first
                # group's k-accumulation), then remaining x chunks.
                # x[0] and W[k=0] go in k-split halves so the very first
                # matmul's operands land as early as possible.
                sync.dma_start(
                    out=x_sb[:, 0 : 2 * P], in_=xh[:, 0 : 2 * P]
                ).then_inc(xs_sem[0], 16)
                sync.dma_start(
                    out=w_sb[:, 0:D_OUT], in_=wh[:, 0:D_OUT]
                ).then_inc(ws_sem[0], 16)
                sync.dma_start(
                    out=x_sb[:, 2 * P : KO * P], in_=xh[:, 2 * P : KO * P]
                ).then_inc(xs0b, 16)
                for k in range(1, KO):
                    sync.dma_start(
                        out=w_sb[:, k * D_OUT : (k + 1) * D_OUT],
                        in_=wh[:, k * D_OUT : (k + 1) * D_OUT],
                    ).then_inc(ws_sem[k], 16)
                for i, (a, b) in enumerate(x_chunks):
                    if i == 0:
                        continue
                    sync.dma_start(
                        out=x_sb[:, a * KO * P : b * KO * P],
                        in_=xh[:, a * KO * P : b * KO * P],
                    ).then_inc(xs_sem[i], 16)
                # output DMAs on this HWDGE ring too (ring is idle by then)
                for i, (a, b) in enumerate(o_chunks):
                    sync.wait_ge(cp0, b)
                    sync.wait_ge(cp1, b)
                    sync.dma_start(
                        out=yh[:, a * D_OUT : b * D_OUT],
                        in_=ot_sb[:, (a % NOB) * D_OUT : (a % NOB + b - a) * D_OUT],
                    ).then_inc(os_sem[i], 16)
                for i in range(len(o_chunks)):
                    sync.wait_ge(os_sem[i], 16)

            @block.tensor
            def _(tensor):
                # HAM warmup: ~10 throwaway matmuls on a scratch bank while the
                # first input DMAs are in flight, so real matmuls start at the
                # un-throttled PE clock (and the wait time isn't dead time).
                if N_WARM:
                    tensor.wait_ge(warm_sem, 1)
                for wi in range(N_WARM):
                    tensor.matmul(
                        ps_warm[:, :NF],
                        lhsT=dummy[:, :P],
                        rhs=dummy[:, :],
                        start=(wi == 0),
                        stop=(wi == N_WARM - 1),
                    )
                for m in range(MT):
                    if m >= NPS:  # psum pair reuse: copies of group m-NPS done
                        tensor.wait_ge(cp0, m - NPS + 1)
                        tensor.wait_ge(cp1, m - NPS + 1)
                    if m == 0 or x_chunk_of[m] != x_chunk_of[m - 1]:
                        tensor.wait_ge(xs_sem[x_chunk_of[m]], 16)
                    for k in range(KO):
                        if m == 0:
                            tensor.wait_ge(ws_sem[k], 16)
                            if k == 2:
                                tensor.wait_ge(xs0b, 16)
                        for n in range(NT):
                            mm = tensor.matmul(
                                ps_tile(m, n),
                                lhsT=x_tile(m, k),
                                rhs=w_tile(k, n),
                                start=(k == 0),
                                stop=(k == KO - 1),
                            )
                            if k == KO - 1:
                                mm.then_inc(mm_sems[n], 1)

            @block.gpsimd
            def _(gpsimd):
                if N_WARM:
                    gpsimd.memset(dummy[:, :], 0).then_inc(warm_sem, 1)

            @block.vector
            def _(vector):
                for m in range(MT):
                    vector.wait_ge(mm0, m + 1)
                    if m >= NOB:
                        vector.wait_ge(os_sem[o_chunk_of[m - NOB]], 16)
                    vector.tensor_copy(ot_tile(m, 0), ps_tile(m, 0)).then_inc(cp0, 1)

            @block.scalar
            def _(scalar):
                for m in range(MT):
                    scalar.wait_ge(mm1, m + 1)
                    if m >= NOB:
                        scalar.wait_ge(os_sem[o_chunk_of[m - NOB]], 16)
                    scalar.copy(ot_tile(m, 1), ps_tile(m, 1)).then_inc(cp1, 1)



    _prog_cache[key] = nc
    return nc


def _build_program(cap, dt_name, split3):
    if IMPL == "raw" and not split3:
        return _build_program_raw(cap, dt_name)
    key = (cap, dt_name, split3)
    if key in _prog_cache:
        return _prog_cache[key]

    import concourse.mybir as mybir
    import concourse.tile as tile
    from concourse import bacc
    from concourse.bass import ds, ts

    mm_dt = getattr(mybir.dt, dt_name)
    f32 = mybir.dt.float32
    P = 128
    NF = 512  # matmul free dim / PSUM bank
    KO = D_IN // P  # 8 contraction tiles
    MT = cap // P  # token tiles
    NT = D_OUT // NF  # 2 output column tiles

    # Host-side packed layouts (one contiguous run per partition per DMA):
    #   xh[p, m*KO*P + k*P + c] = X[m*P + c, k*P + p]   (lhsT tiles, row-major)
    #   wh[p, k*D_OUT + n]      = W[k*P + p, n]
    #   yh[p, m*D_OUT + n]      = Y[m*P + p, n]
    nc = bacc.Bacc("TRN2", target_bir_lowering=False, debug=False)
    xh = nc.dram_tensor("xh", [P, MT * KO * P], mm_dt, kind="ExternalInput").ap()
    wh = nc.dram_tensor("wh", [P, KO * D_OUT], mm_dt, kind="ExternalInput").ap()
    if split3:
        xhl = nc.dram_tensor("xhl", [P, MT * KO * P], mm_dt, kind="ExternalInput").ap()
        whl = nc.dram_tensor("whl", [P, KO * D_OUT], mm_dt, kind="ExternalInput").ap()
    yh = nc.dram_tensor("yh", [P, MT * D_OUT], f32, kind="ExternalOutput").ap()

    with tile.TileContext(nc) as tc:
        with (
            tc.tile_pool(name="const", bufs=1) as cpool,
            tc.tile_pool(name="psum", bufs=3, space="PSUM") as ppool,
            tc.tile_pool(name="outp", bufs=4) as opool,
        ):
            w_sb = cpool.tile([P, KO * D_OUT], mm_dt, tag="w")
            x_sb = cpool.tile([P, MT, KO * P], mm_dt, tag="x")
            if split3:
                wl_sb = cpool.tile([P, KO * D_OUT], mm_dt, tag="wl")
                xl_sb = cpool.tile([P, MT, KO * P], mm_dt, tag="xl")
            # x[m=0] first (first matmul group), then W per-k (pipelines the
            # first group's k-accumulation), then the remaining x tiles.
            nc.sync.dma_start(out=x_sb[:, 0, :], in_=xh[:, ds(0, KO * P)])
            if split3:
                nc.sync.dma_start(out=xl_sb[:, 0, :], in_=xhl[:, ds(0, KO * P)])
            for k in range(KO):
                nc.sync.dma_start(
                    out=w_sb[:, ds(k * D_OUT, D_OUT)], in_=wh[:, ds(k * D_OUT, D_OUT)]
                )
                if split3:
                    nc.sync.dma_start(
                        out=wl_sb[:, ds(k * D_OUT, D_OUT)],
                        in_=whl[:, ds(k * D_OUT, D_OUT)],
                    )
            for m in range(1, MT):
                nc.sync.dma_start(
                    out=x_sb[:, m, :], in_=xh[:, ds(m * KO * P, KO * P)]
                )
                if split3:
                    nc.sync.dma_start(
                        out=xl_sb[:, m, :], in_=xhl[:, ds(m * KO * P, KO * P)]
                    )

            for m in range(MT):
                ps = [
                    ppool.tile([P, NF], f32, tag=f"ps{n}", name=f"ps{n}_{m}")
                    for n in range(NT)
                ]
                for k in range(KO):
                    pairs = [(x_sb, w_sb)]
                    if split3:
                        pairs = [(x_sb, w_sb), (x_sb, wl_sb), (xl_sb, w_sb)]
                    for pi, (xs, ws) in enumerate(pairs):
                        for n in range(NT):
                            nc.tensor.matmul(
                                ps[n][:],
                                lhsT=xs[:, m, ds(k * P, P)],
                                rhs=ws[:, ds(k * D_OUT + n * NF, NF)],
                                start=(k == 0 and pi == 0),
                                stop=(k == KO - 1 and pi == len(pairs) - 1),
                            )
                ot = opool.tile([P, D_OUT], f32, tag="ot")
                for n in range(NT):
                    if n % 2 == 0:
                        nc.vector.tensor_copy(ot[:, ds(n * NF, NF)], ps[n][:])
                    else:
                        nc.scalar.copy(ot[:, ds(n * NF, NF)], ps[n][:])
                nc.gpsimd.dma_start(out=yh[:, ds(m * D_OUT, D_OUT)], in_=ot[:])
    nc.compile()
    _prog_cache[key] = nc
    return nc


def _route(x, Wr, br):
    """fp32 router identical to the reference's routing decisions."""
    logits = x @ Wr.T + br  # [N, E] fp32
    m = logits.max(axis=-1, keepdims=True)
    e = np.exp(logits - m)
    probs = e / e.sum(axis=-1, keepdims=True)  # fp32
    # descending stable sort == jax.lax.top_k tie-breaking (lowest index wins)
    top_i = np.argsort(-probs, axis=-1, kind="stable")[:, :TOP_K]
    top_w = np.take_along_axis(probs, top_i, axis=-1)
    top_w = top_w / top_w.sum(axis=-1, keepdims=True)
    mean_prob = probs.mean(axis=0, dtype=np.float64)
    aux = np.float32(np.mean((mean_prob - 1.0 / N_EXPERTS) ** 2) * LB_WEIGHT)
    return probs, top_i, top_w, aux


def kernel(x, W, b, Wr, br):
    global last_results
    x = np.ascontiguousarray(np.asarray(x, dtype=np.float32))
    W = np.asarray(W, dtype=np.float32)
    b = np.asarray(b, dtype=np.float32)
    Wr = np.asarray(Wr, dtype=np.float32)
    br = np.asarray(br, dtype=np.float32)

    probs, top_i, top_w, aux = _route(x, Wr, br)

    # --- dispatch: gather tokens per expert, pad to capacity ---
    np_dt = _np_dtype(MM_DTYPE)
    idx = [np.where((top_i == e).any(axis=1))[0] for e in range(N_EXPERTS)]
    counts = np.array([len(i) for i in idx])
    cap = max(CAP_MIN, int(-(-counts.max() // 128) * 128))

    P = 128
    KO = D_IN // P
    MT = cap // P

    def pack_x(xg):  # [cap, D_IN] -> [P, MT*KO*P]: xh[p, (m,k,c)] = xg[mP+c, kP+p]
        return np.ascontiguousarray(
            xg.reshape(MT, P, KO, P).transpose(3, 0, 2, 1).reshape(P, MT * KO * P)
        )

    def pack_w(we):  # [D_IN, D_OUT] -> [P, KO*D_OUT]: wh[p, (k,n)] = we[kP+p, n]
        return np.ascontiguousarray(
            we.reshape(KO, P, D_OUT).transpose(1, 0, 2).reshape(P, KO * D_OUT)
        )

    in_maps = []
    for e in range(N_EXPERTS):
        xg = np.zeros((cap, D_IN), dtype=np.float32)
        xg[: counts[e]] = x[idx[e]]
        m = {"xh": pack_x(xg.astype(np_dt)), "wh": pack_w(W[e].astype(np_dt))}
        if SPLIT3:
            xlo = (xg - xg.astype(np_dt).astype(np.float32)).astype(np_dt)
            wlo = (W[e] - W[e].astype(np_dt).astype(np.float32)).astype(np_dt)
            m["xhl"] = pack_x(xlo)
            m["whl"] = pack_w(wlo)
        in_maps.append(m)

    # --- device: Y_e = X_e @ W[e] on core e ---
    from concourse.bass_utils import run_bass_kernel_spmd

    nc = _build_program(cap, MM_DTYPE, SPLIT3)
    trace = os.environ.get("BASS_MOE_TRACE", "0") == "1"
    kwargs = {}
    if trace:
        kwargs = dict(trace=True, tmpdir=os.environ.get("BASS_MOE_TRACE_DIR"))
    res = run_bass_kernel_spmd(nc, in_maps, list(range(N_EXPERTS)), **kwargs)
    last_results = res
    # yh[p, (m,n)] = Y[m*P+p, n]  ->  Y[cap, D_OUT]
    Y = np.stack(
        [
            res.results[i]["yh"]
            .reshape(P, MT, D_OUT)
            .transpose(1, 0, 2)
            .reshape(cap, D_OUT)
            for i in range(N_EXPERTS)
        ]
    )  # [E, cap, O] f32

    # --- combine: out[n] = sum_k w_k * (Y[e_k, pos_k] + b[e_k]) ---
    pos = np.zeros((N_EXPERTS, N_TOKENS), dtype=np.int64)
    for e in range(N_EXPERTS):
        pos[e, idx[e]] = np.arange(counts[e])
    tok = np.arange(N_TOKENS)
    out = np.zeros((N_TOKENS, D_OUT), dtype=np.float32)
    for k in range(TOP_K):
        ek = top_i[:, k]
        rows = Y[ek, pos[ek, tok]]  # [N, O]
        out += top_w[:, k : k + 1] * (rows + b[ek])

    return out, aux
